# revision 2
# baseline (speedup 1.0000x reference)
"""Cross-attention Trainium2 kernel v2 (nn_CrossAttention_7627861918199).

Sharding: 8 cores = (batch b, head-half hh); core c handles batch c//2 and
heads [4*(c%2), 4*(c%2)+4) for ALL 1024 queries, then pairs (2b, 2b+1)
exchange attention outputs (AllGather over DRAM) so each core applies the
output projection for its own 512-query half.

SPMD trick: the program is core-independent. Host arranges per-core inputs:
  - x query slots: slot A (cols 0:512) = the OTHER core's query half
    (computed first, exchanged early), slot B = own half.
  - Wo inner chunks permuted to [my p0, my p1, partner p0, partner p1].
  - m0/m1 one-hot [128,1] selectors pick the partner slot out of the
    AllGather result.

Per-core pipeline (all bf16 matmuls, f32 psum):
  qT = Wq_sh @ xT ; kT = Wk_sh @ ctxT ; v = ctx @ Wv_sh^T (+ ones col)
  4 virtual head-pair passes (pair p, query slot sl):
    S^T chunks [128kv, 2, 512] = k @ qT -> exp (ACT, scale=1/8) -> P^T bf16
    O^T_aug[65, 512] += v_aug^T @ P^T  (psum, 32-chunk accumulation)
    evict av psum -> sbuf, reciprocal_approx_fast on denom row,
    PE ones-matmul broadcasts recip, o = av * rec -> o_sb
  after slot-A passes: AllGather O^T pair chunk with partner, mask-select.
  y accumulated in sbuf f32: y_qc = sum_ic O^T_ic^T @ WoT_ic (+ bo).
"""

import numpy as np
import ml_dtypes

HEADS = 8
DIM_HEAD = 64
D = 512
B, NQ, NKV = 4, 1024, 4096
N_CORES = 8
P = 128
DC = 4                      # contraction chunks of 128 over D
NCHUNK = NKV // P           # 32 kv chunks
NT = NKV // 512             # 8 kv 512-tiles
LH = 4                      # local heads
NQH = 512                   # queries per slot
BF16 = ml_dtypes.bfloat16

_PROGRAMS = {}

CC_GROUPS = [[0, 1], [2, 3], [4, 5], [6, 7]]


def _build(need_mask: bool, num_devices: int = N_CORES):
    import concourse.mybir as mybir
    import concourse.tile as tile
    from concourse import bacc

    dt = mybir.dt
    f32, bf = dt.float32, dt.bfloat16

    nc = bacc.Bacc("TRN2", target_bir_lowering=False, debug=False,
                   num_devices=num_devices)

    xT = nc.dram_tensor("xT", [D, NQ], bf, kind="ExternalInput").ap()
    ctxT = nc.dram_tensor("ctxT", [D, NKV], bf, kind="ExternalInput").ap()
    wqT = nc.dram_tensor("wqT", [D, 256], bf, kind="ExternalInput").ap()
    wkT = nc.dram_tensor("wkT", [D, 256], bf, kind="ExternalInput").ap()
    wvT = nc.dram_tensor("wvT", [D, 256], bf, kind="ExternalInput").ap()
    woT = nc.dram_tensor("woT", [D, D], bf, kind="ExternalInput").ap()
    bo = nc.dram_tensor("bo", [1, D], f32, kind="ExternalInput").ap()
    msel = nc.dram_tensor("msel", [P, 2], f32, kind="ExternalInput").ap()
    if need_mask:
        maskb = nc.dram_tensor("maskb", [P, NCHUNK], f32,
                               kind="ExternalInput").ap()
    y = nc.dram_tensor("y", [NQH, D], f32, kind="ExternalOutput").ap()

    Exp = mybir.ActivationFunctionType.Exp

    with tile.TileContext(nc) as tc:
        with tc.tile_pool(name="big", bufs=1) as big, \
             tc.tile_pool(name="work", bufs=2) as work, \
             tc.tile_pool(name="pTp", bufs=4) as pTp, \
             tc.tile_pool(name="dram", bufs=2, space="DRAM") as dram, \
             tc.tile_pool(name="proj_ps", bufs=2, space="PSUM") as proj_ps, \
             tc.tile_pool(name="sc_ps", bufs=2, space="PSUM") as sc_ps, \
             tc.tile_pool(name="av_ps", bufs=1, space="PSUM") as av_ps:

            ctx_sb = big.tile([P, DC, NKV], bf, name="ctx_sb")
            x_sb = big.tile([P, DC, NQ], bf, name="x_sb")
            wq_sb = big.tile([P, DC, 256], bf, name="wq_sb")
            wk_sb = big.tile([P, DC, 256], bf, name="wk_sb")
            wv_sb = big.tile([P, DC, 256], bf, name="wv_sb")
            wo_sb = big.tile([P, DC, D], bf, name="wo_sb")
            bo_bc = big.tile([P, D], f32, name="bo_bc")
            q_sb = big.tile([P, 2, NQ], bf, name="q_sb")
            k_sb = big.tile([P, 2, NKV], bf, name="k_sb")
            v_sb = big.tile([P, NCHUNK, LH, DIM_HEAD + 1], bf, name="v_sb")
            o_sb = big.tile([P, 2, NQ], bf, name="o_sb")
            po_sb = big.tile([P, 2, NQH], bf, name="po_sb")
            y_acc = big.tile([P, DC, D], f32, name="y_acc")
            ones_sb = big.tile([P, DIM_HEAD], bf, name="ones_sb")
            msel_sb = big.tile([P, 2], f32, name="msel_sb")
            if need_mask:
                mb_sb = big.tile([P, NCHUNK], f32, name="mb_sb")

            # ---- input DMAs: critical tiles first on each queue ----
            xTr = xT.rearrange("(c p) n -> p c n", p=P)
            wqr = wqT.rearrange("(c p) n -> p c n", p=P)
            wkr = wkT.rearrange("(c p) n -> p c n", p=P)
            wvr = wvT.rearrange("(c p) n -> p c n", p=P)
            wor = woT.rearrange("(c p) n -> p c n", p=P)
            ctxr = ctxT.rearrange("(c p) n -> p c n", p=P)

            def ctx_dma(eng, nt):
                eng.dma_start(ctx_sb[:, :, nt * 512:(nt + 1) * 512],
                              ctxr[:, :, nt * 512:(nt + 1) * 512])

            # critical tiles first; bulk gated behind them so the DMA
            # engines don't interleave the 5MB tail into the critical 1.5MB
            nc.gpsimd.dma_start(wk_sb[:], wkr)
            ctx_dma(nc.sync, 0)
            nc.gpsimd.dma_start(wq_sb[:], wqr)
            nc.sync.dma_start(x_sb[:, :, 0:NQH], xTr[:, :, 0:NQH])
            nc.gpsimd.dma_start(wv_sb[:], wvr)
            ctx_dma(nc.gpsimd, 1)
            ctx_dma(nc.sync, 2)
            ctx_dma(nc.gpsimd, 3)
            ctx_dma(nc.sync, 4)
            ctx_dma(nc.gpsimd, 5)
            ctx_dma(nc.sync, 6)
            ctx_dma(nc.gpsimd, 7)
            nc.sync.dma_start(x_sb[:, :, NQH:NQ], xTr[:, :, NQH:NQ])
            nc.gpsimd.dma_start(wo_sb[:], wor)
            nc.gpsimd.dma_start(bo_bc[:], bo.to_broadcast([P, D]))
            nc.gpsimd.dma_start(msel_sb[:], msel[:])
            if need_mask:
                nc.gpsimd.dma_start(mb_sb[:], maskb[:])
            nc.vector.memset(v_sb[:, :, :, DIM_HEAD], 1.0)
            nc.vector.memset(ones_sb[:], 1.0)

            # ---- projection emitters ----
            def emit_m1(p, nt):
                ps = proj_ps.tile([P, 512], f32, name="ps_proj", tag="proj")
                for kc in range(DC):
                    nc.tensor.matmul(
                        ps, wk_sb[:, kc, p * P:(p + 1) * P],
                        ctx_sb[:, kc, nt * 512:(nt + 1) * 512],
                        start=(kc == 0), stop=(kc == DC - 1))
                nc.vector.tensor_copy(
                    out=k_sb[:, p, nt * 512:(nt + 1) * 512], in_=ps)

            def emit_m0(p, sl):
                ps = proj_ps.tile([P, 512], f32, name="ps_proj", tag="proj")
                for kc in range(DC):
                    nc.tensor.matmul(
                        ps, wq_sb[:, kc, p * P:(p + 1) * P],
                        x_sb[:, kc, sl * NQH:(sl + 1) * NQH],
                        start=(kc == 0), stop=(kc == DC - 1))
                nc.vector.tensor_copy(
                    out=q_sb[:, p, sl * NQH:(sl + 1) * NQH], in_=ps)

            def emit_v(j):
                ps = proj_ps.tile([P, 512], f32, name="ps_proj", tag="proj")
                for kc in range(DC):
                    nc.tensor.matmul(
                        ps[:, 0:256], ctx_sb[:, kc, j * P:(j + 1) * P],
                        wv_sb[:, kc, :],
                        start=(kc == 0), stop=(kc == DC - 1))
                nc.vector.tensor_copy(
                    out=v_sb[:, j, :, 0:DIM_HEAD],
                    in_=ps[:, 0:256].rearrange("p (h d) -> p h d", h=LH))

            # ---- M5 contributions: y_acc[:, qc, :] += O^T_ic^T @ WoT_ic ----
            m5_count = [0] * DC

            def emit_m5(contribs):
                for qc in range(DC):
                    ps = proj_ps.tile([P, 512], f32, name="ps_proj",
                                      tag="proj")
                    for i, (o_ap, ic) in enumerate(contribs):
                        nc.tensor.matmul(
                            ps, o_ap[:, qc * P:(qc + 1) * P],
                            wo_sb[:, ic, :],
                            start=(i == 0), stop=(i == len(contribs) - 1))
                    if m5_count[qc] == 0:
                        nc.vector.tensor_add(y_acc[:, qc, :], ps, bo_bc)
                    else:
                        nc.vector.tensor_add(y_acc[:, qc, :],
                                             y_acc[:, qc, :], ps)
                    m5_count[qc] += len(contribs)
                    if m5_count[qc] == DC:
                        eng = nc.sync if qc % 2 == 0 else nc.gpsimd
                        eng.dma_start(y[qc * P:(qc + 1) * P, :],
                                      y_acc[:, qc, :])

            # ---- partner-slot select, deferred past CC latency ----
            ex_outs = [None, None]

            def emit_select(p):
                exrd = work.tile([P, 2, NQH], bf, name="exrd", tag="exrd")
                nc.sync.dma_start(
                    exrd[:], ex_outs[p][:].rearrange("s p n -> p s n"))
                tsel = work.tile([P, NQH], bf, name="tsel", tag="tsel")
                nc.vector.tensor_scalar_mul(
                    tsel[:], exrd[:, 0, :], msel_sb[:, 0, None])
                nc.vector.tensor_scalar_mul(
                    po_sb[:, p, :], exrd[:, 1, :], msel_sb[:, 1, None])
                nc.vector.tensor_add(po_sb[:, p, :], po_sb[:, p, :],
                                     tsel[:])

            # ---- normalize split: 1/denom on ACT as exp(-ln d), read
            # straight from av psum (keeps the slow DVE reciprocal out of
            # every engine's watermark chain); PE ones-broadcast + mul +
            # DMAs deferred into the next vi once rec is ready ----
            Ln = mybir.ActivationFunctionType.Ln

            def emit_recip(av_ps_tile, rec_bf, parity):
                ln_t = work.tile([P, 2, 512], f32, name="ln_t", tag="lnt",
                                 bufs=2)
                nc.scalar.activation(
                    ln_t[DIM_HEAD:DIM_HEAD + 1, parity, :],
                    av_ps_tile[DIM_HEAD:DIM_HEAD + 1, :], Ln)
                nc.scalar.activation(
                    rec_bf[DIM_HEAD:DIM_HEAD + 1, parity, :],
                    ln_t[DIM_HEAD:DIM_HEAD + 1, parity, :], Exp, scale=-1.0)

            def emit_finish(st):
                av_sb, rec_bf, p, sl = st
                for parity in (0, 1):
                    bc = proj_ps.tile([P, 512], f32, name="ps_proj",
                                      tag="proj")
                    nc.tensor.matmul(
                        bc[0:DIM_HEAD, :],
                        ones_sb[DIM_HEAD:DIM_HEAD + 1, :],
                        rec_bf[DIM_HEAD:DIM_HEAD + 1, parity, :],
                        start=True, stop=True)
                    o_tmp = work.tile([P, 512], bf, name="o_tmp", tag="otmp")
                    nc.vector.tensor_mul(o_tmp[0:DIM_HEAD, :],
                                         av_sb[0:DIM_HEAD, parity, :],
                                         bc[0:DIM_HEAD, :])
                    nc.sync.dma_start(
                        o_sb[parity * DIM_HEAD:(parity + 1) * DIM_HEAD, p,
                             sl * NQH:(sl + 1) * NQH],
                        o_tmp[0:DIM_HEAD, :])
                if sl == 0:
                    # exchange this pair's slot-A O with the partner core
                    ex_in = dram.tile([P, NQH], bf, name="ex_in", tag="exin")
                    nc.sync.dma_start(ex_in[:], o_sb[:, p, 0:NQH])
                    ex_out = dram.tile([2, P, NQH], bf, name="ex_out",
                                       tag="exout")
                    ex_outs[p] = ex_out
                    nc.gpsimd.collective_compute(
                        "AllGather", mybir.AluOpType.bypass,
                        replica_groups=CC_GROUPS,
                        ins=[ex_in.opt()], outs=[ex_out.opt()])

            # ---- virtual head-pair passes ----
            SKEW = 2
            norm_pend = []
            for vi, (p, sl) in enumerate([(0, 0), (1, 0), (0, 1), (1, 1)]):
                av0 = av_ps.tile([DIM_HEAD + 1, 512], f32, name="av0",
                                 tag="av0")
                av1 = av_ps.tile([DIM_HEAD + 1, 512], f32, name="av1",
                                 tag="av1")
                pend = []

                def emit_av(j, pT):
                    nc.tensor.matmul(
                        av0[:, :], v_sb[:, j, 2 * p, :], pT[:, 0, :],
                        start=(j == 0), stop=(j == NCHUNK - 1))
                    nc.tensor.matmul(
                        av1[:, :], v_sb[:, j, 2 * p + 1, :], pT[:, 1, :],
                        start=(j == 0), stop=(j == NCHUNK - 1))

                for j in range(NCHUNK):
                    # previous vi's normalize finish, past the boundary
                    if j == 5 and norm_pend:
                        emit_finish(norm_pend.pop(0))
                    # projection interleave (front-loaded into vi 0/1)
                    if vi == 0:
                        if j == 0:
                            emit_m1(0, 0)
                            emit_m0(0, 0)
                        elif j == 1:
                            emit_m1(0, 1)
                        elif j % 4 == 0 and 4 <= j <= 24:
                            emit_m1(0, j // 4 + 1)
                        elif j % 4 == 2:
                            emit_m1(1, j // 4)
                        if j == 9:
                            emit_m0(1, 0)
                        emit_v(j)
                    elif vi == 1:
                        if j == 9:
                            emit_m0(0, 1)
                        elif j == 19:
                            emit_m0(1, 1)
                    elif vi == 3:
                        # selects sit past the ~60us CC horizon so the
                        # in-order DVE never blocks on the collective
                        if j == 0:
                            emit_select(0)
                        elif j == 4:
                            emit_m5([(po_sb[:, 0, :], 2)])   # partner p0
                        elif j == 12:
                            emit_m5([(o_sb[:, 0, NQH:NQ], 0)])  # own p0

                    sc = sc_ps.tile([P, 2, 512], f32, name="sc", tag="sc")
                    nc.tensor.matmul(
                        sc[:, 0, :],
                        k_sb[0:DIM_HEAD, p, j * P:(j + 1) * P],
                        q_sb[0:DIM_HEAD, p, sl * NQH:(sl + 1) * NQH],
                        start=True, stop=True)
                    nc.tensor.matmul(
                        sc[:, 1, :],
                        k_sb[DIM_HEAD:P, p, j * P:(j + 1) * P],
                        q_sb[DIM_HEAD:P, p, sl * NQH:(sl + 1) * NQH],
                        start=True, stop=True)
                    pT = pTp.tile([P, 2, 512], bf, name="pT", tag="pT")
                    if need_mask:
                        nc.scalar.activation(
                            pT[:], sc[:], Exp,
                            bias=mb_sb[:, j, None], scale=0.125)
                    else:
                        nc.scalar.activation(pT[:], sc[:], Exp, scale=0.125)
                    pend.append((j, pT))
                    if len(pend) > SKEW:
                        emit_av(*pend.pop(0))
                for item in pend:
                    emit_av(*item)

                # ---- vi tail: 1/denom on ACT from psum, evict av psum ----
                rec_bf = work.tile([P, 2, 512], bf, name="rec_bf",
                                   tag="recbf", bufs=2)
                emit_recip(av0, rec_bf, 0)
                emit_recip(av1, rec_bf, 1)
                av_sb = work.tile([P, 2, 512], f32, name="av_sb", tag="avsb")
                nc.vector.tensor_copy(out=av_sb[0:DIM_HEAD, 0, :],
                                      in_=av0[0:DIM_HEAD, :])
                nc.vector.tensor_copy(out=av_sb[0:DIM_HEAD, 1, :],
                                      in_=av1[0:DIM_HEAD, :])
                norm_pend.append((av_sb, rec_bf, p, sl))

            # tail: last vi's normalize, partner-p1 select, fused final M5
            emit_finish(norm_pend.pop(0))
            emit_select(1)
            emit_m5([(o_sb[:, 1, NQH:NQ], 1), (po_sb[:, 1, :], 3)])

    nc.compile()
    return nc


def _get_program(need_mask: bool):
    if need_mask not in _PROGRAMS:
        _PROGRAMS[need_mask] = _build(need_mask)
    return _PROGRAMS[need_mask]


def _prep_inputs(x, context, mask, Wq, Wkv, Wo, bo):
    """Host-side shard + transpose + cast. Returns list of per-core in_maps."""
    x = np.asarray(x, dtype=np.float32)
    context = np.asarray(context, dtype=np.float32)
    mask = np.asarray(mask)
    Wq = np.asarray(Wq, dtype=np.float32)
    Wkv = np.asarray(Wkv, dtype=np.float32)
    Wo = np.asarray(Wo, dtype=np.float32)
    bo = np.asarray(bo, dtype=np.float32)

    need_mask = not bool(mask.all())
    WqT = np.ascontiguousarray(Wq.T).astype(BF16)      # [D, inner]
    WkT = np.ascontiguousarray(Wkv[:D].T).astype(BF16)
    WvT = np.ascontiguousarray(Wkv[D:].T).astype(BF16)
    WoT = np.ascontiguousarray(Wo.T).astype(np.float32)  # [inner, out]
    bo2 = bo.reshape(1, D)

    ctxTs = [np.ascontiguousarray(context[b].T).astype(BF16)
             for b in range(B)]
    if need_mask:
        mb = [np.where(mask[b], 0.0, -1e30).astype(np.float32)
              .reshape(NCHUNK, P).T.copy() for b in range(B)]

    in_maps = []
    for c in range(N_CORES):
        b, hh = divmod(c, 2)
        other, own = (1 - hh) * NQH, hh * NQH
        xa = np.concatenate(
            [x[b, other:other + NQH, :], x[b, own:own + NQH, :]], axis=0)
        # Wo inner chunks permuted: [my 256 rows, partner 256 rows]
        woTa = np.concatenate(
            [WoT[hh * 256:(hh + 1) * 256, :],
             WoT[(1 - hh) * 256:(2 - hh) * 256, :]], axis=0).astype(BF16)
        msel = np.zeros((P, 2), np.float32)
        msel[:, 1 - hh] = 1.0   # pick partner slot from AllGather
        m = {
            "xT": np.ascontiguousarray(xa.T).astype(BF16),
            "ctxT": ctxTs[b],
            "wqT": np.ascontiguousarray(WqT[:, hh * 256:(hh + 1) * 256]),
            "wkT": np.ascontiguousarray(WkT[:, hh * 256:(hh + 1) * 256]),
            "wvT": np.ascontiguousarray(WvT[:, hh * 256:(hh + 1) * 256]),
            "woT": woTa,
            "bo": bo2,
            "msel": msel,
        }
        if need_mask:
            m["maskb"] = mb[b]
        in_maps.append(m)
    return in_maps, need_mask


def run_sharded(inputs, trace=False):
    """Run on 8 cores; returns (full_output, BassKernelResults)."""
    from concourse import bass_utils
    in_maps, need_mask = _prep_inputs(**inputs)
    nc = _get_program(need_mask)
    res = bass_utils.run_bass_kernel_spmd(
        nc, in_maps, core_ids=list(range(N_CORES)), trace=trace)
    out = np.empty((B, NQ, D), dtype=np.float32)
    for c in range(N_CORES):
        b, hh = divmod(c, 2)
        out[b, hh * NQH:(hh + 1) * NQH, :] = res.results[c]["y"]
    return out, res


def kernel(**inputs) -> np.ndarray:
    out, _ = run_sharded(inputs, trace=False)
    return out


# revision 3
# speedup vs baseline: 1.0539x; 1.0539x over previous
"""Cross-attention Trainium2 kernel v2 (nn_CrossAttention_7627861918199).

Sharding: 8 cores = (batch b, head-half hh); core c handles batch c//2 and
heads [4*(c%2), 4*(c%2)+4) for ALL 1024 queries, then pairs (2b, 2b+1)
exchange attention outputs (AllGather over DRAM) so each core applies the
output projection for its own 512-query half.

SPMD trick: the program is core-independent. Host arranges per-core inputs:
  - x query slots: slot A (cols 0:512) = the OTHER core's query half
    (computed first, exchanged early), slot B = own half.
  - Wo inner chunks permuted to [my p0, my p1, partner p0, partner p1].
  - m0/m1 one-hot [128,1] selectors pick the partner slot out of the
    AllGather result.

Per-core pipeline (all bf16 matmuls, f32 psum):
  qT = Wq_sh @ xT ; kT = Wk_sh @ ctxT ; v = ctx @ Wv_sh^T (+ ones col)
  4 virtual head-pair passes (pair p, query slot sl):
    S^T chunks [128kv, 2, 512] = k @ qT -> exp (ACT, scale=1/8) -> P^T bf16
    O^T_aug[65, 512] += v_aug^T @ P^T  (psum, 32-chunk accumulation)
    evict av psum -> sbuf, reciprocal_approx_fast on denom row,
    PE ones-matmul broadcasts recip, o = av * rec -> o_sb
  after slot-A passes: AllGather O^T pair chunk with partner, mask-select.
  y accumulated in sbuf f32: y_qc = sum_ic O^T_ic^T @ WoT_ic (+ bo).
"""

import numpy as np
import ml_dtypes

HEADS = 8
DIM_HEAD = 64
D = 512
B, NQ, NKV = 4, 1024, 4096
N_CORES = 8
P = 128
DC = 4                      # contraction chunks of 128 over D
NCHUNK = NKV // P           # 32 kv chunks
NT = NKV // 512             # 8 kv 512-tiles
LH = 4                      # local heads
NQH = 512                   # queries per slot
BF16 = ml_dtypes.bfloat16

_PROGRAMS = {}

CC_GROUPS = [[0, 1], [2, 3], [4, 5], [6, 7]]


def _build(need_mask: bool, num_devices: int = N_CORES):
    import concourse.mybir as mybir
    import concourse.tile as tile
    from concourse import bacc

    dt = mybir.dt
    f32, bf = dt.float32, dt.bfloat16

    nc = bacc.Bacc("TRN2", target_bir_lowering=False, debug=False,
                   num_devices=num_devices)

    xT = nc.dram_tensor("xT", [D, NQ], bf, kind="ExternalInput").ap()
    ctxT = nc.dram_tensor("ctxT", [D, NKV], bf, kind="ExternalInput").ap()
    wqT = nc.dram_tensor("wqT", [D, 256], bf, kind="ExternalInput").ap()
    wkT = nc.dram_tensor("wkT", [D, 256], bf, kind="ExternalInput").ap()
    wvT = nc.dram_tensor("wvT", [D, 256], bf, kind="ExternalInput").ap()
    woT = nc.dram_tensor("woT", [D, D], bf, kind="ExternalInput").ap()
    bo = nc.dram_tensor("bo", [1, D], f32, kind="ExternalInput").ap()
    msel = nc.dram_tensor("msel", [P, 2], f32, kind="ExternalInput").ap()
    if need_mask:
        maskb = nc.dram_tensor("maskb", [P, NCHUNK], f32,
                               kind="ExternalInput").ap()
    y = nc.dram_tensor("y", [NQH, D], f32, kind="ExternalOutput").ap()

    Exp = mybir.ActivationFunctionType.Exp

    with tile.TileContext(nc) as tc:
        with tc.tile_pool(name="big", bufs=1) as big, \
             tc.tile_pool(name="work", bufs=2) as work, \
             tc.tile_pool(name="pTp", bufs=6) as pTp, \
             tc.tile_pool(name="dram", bufs=2, space="DRAM") as dram, \
             tc.tile_pool(name="proj_ps", bufs=2, space="PSUM") as proj_ps, \
             tc.tile_pool(name="sc_ps", bufs=2, space="PSUM") as sc_ps, \
             tc.tile_pool(name="av_ps", bufs=1, space="PSUM") as av_ps:

            ctx_sb = big.tile([P, DC, NKV], bf, name="ctx_sb")
            x_sb = big.tile([P, DC, NQ], bf, name="x_sb")
            wq_sb = big.tile([P, DC, 256], bf, name="wq_sb")
            wk_sb = big.tile([P, DC, 256], bf, name="wk_sb")
            wv_sb = big.tile([P, DC, 256], bf, name="wv_sb")
            wo_sb = big.tile([P, DC, D], bf, name="wo_sb")
            bo_bc = big.tile([P, D], f32, name="bo_bc")
            q_sb = big.tile([P, 2, NQ], bf, name="q_sb")
            k_sb = big.tile([P, 2, NKV], bf, name="k_sb")
            v_sb = big.tile([P, NCHUNK, LH, DIM_HEAD + 1], bf, name="v_sb")
            o_sb = big.tile([P, 2, NQ], bf, name="o_sb")
            po_sb = big.tile([P, 2, NQH], bf, name="po_sb")
            y_acc = big.tile([P, DC, D], f32, name="y_acc")
            ones_sb = big.tile([P, DIM_HEAD], bf, name="ones_sb")
            msel_sb = big.tile([P, 2], f32, name="msel_sb")
            if need_mask:
                mb_sb = big.tile([P, NCHUNK], f32, name="mb_sb")

            # ---- input DMAs: critical tiles first on each queue ----
            xTr = xT.rearrange("(c p) n -> p c n", p=P)
            wqr = wqT.rearrange("(c p) n -> p c n", p=P)
            wkr = wkT.rearrange("(c p) n -> p c n", p=P)
            wvr = wvT.rearrange("(c p) n -> p c n", p=P)
            wor = woT.rearrange("(c p) n -> p c n", p=P)
            ctxr = ctxT.rearrange("(c p) n -> p c n", p=P)

            def ctx_dma(eng, nt):
                eng.dma_start(ctx_sb[:, :, nt * 512:(nt + 1) * 512],
                              ctxr[:, :, nt * 512:(nt + 1) * 512])

            # critical tiles first; bulk gated behind them so the DMA
            # engines don't interleave the 5MB tail into the critical 1.5MB
            nc.gpsimd.dma_start(wk_sb[:], wkr)
            ctx_dma(nc.sync, 0)
            nc.gpsimd.dma_start(wq_sb[:], wqr)
            nc.sync.dma_start(x_sb[:, :, 0:NQH], xTr[:, :, 0:NQH])
            nc.gpsimd.dma_start(wv_sb[:], wvr)
            ctx_dma(nc.gpsimd, 1)
            ctx_dma(nc.sync, 2)
            ctx_dma(nc.gpsimd, 3)
            ctx_dma(nc.sync, 4)
            ctx_dma(nc.gpsimd, 5)
            ctx_dma(nc.sync, 6)
            ctx_dma(nc.gpsimd, 7)
            nc.sync.dma_start(x_sb[:, :, NQH:NQ], xTr[:, :, NQH:NQ])
            nc.gpsimd.dma_start(wo_sb[:], wor)
            nc.gpsimd.dma_start(bo_bc[:], bo.to_broadcast([P, D]))
            nc.gpsimd.dma_start(msel_sb[:], msel[:])
            if need_mask:
                nc.gpsimd.dma_start(mb_sb[:], maskb[:])
            nc.vector.memset(v_sb[:, :, :, DIM_HEAD], 1.0)
            nc.vector.memset(ones_sb[:], 1.0)

            # ---- projection emitters ----
            def emit_m1(p, nt):
                ps = proj_ps.tile([P, 512], f32, name="ps_proj", tag="proj")
                for kc in range(DC):
                    nc.tensor.matmul(
                        ps, wk_sb[:, kc, p * P:(p + 1) * P],
                        ctx_sb[:, kc, nt * 512:(nt + 1) * 512],
                        start=(kc == 0), stop=(kc == DC - 1))
                nc.vector.tensor_copy(
                    out=k_sb[:, p, nt * 512:(nt + 1) * 512], in_=ps)

            def emit_m0(p, sl):
                ps = proj_ps.tile([P, 512], f32, name="ps_proj", tag="proj")
                for kc in range(DC):
                    nc.tensor.matmul(
                        ps, wq_sb[:, kc, p * P:(p + 1) * P],
                        x_sb[:, kc, sl * NQH:(sl + 1) * NQH],
                        start=(kc == 0), stop=(kc == DC - 1))
                nc.vector.tensor_copy(
                    out=q_sb[:, p, sl * NQH:(sl + 1) * NQH], in_=ps)

            def emit_v(j):
                ps = proj_ps.tile([P, 512], f32, name="ps_proj", tag="proj")
                for kc in range(DC):
                    nc.tensor.matmul(
                        ps[:, 0:256], ctx_sb[:, kc, j * P:(j + 1) * P],
                        wv_sb[:, kc, :],
                        start=(kc == 0), stop=(kc == DC - 1))
                nc.vector.tensor_copy(
                    out=v_sb[:, j, :, 0:DIM_HEAD],
                    in_=ps[:, 0:256].rearrange("p (h d) -> p h d", h=LH))

            # ---- M5 contributions: y_acc[:, qc, :] += O^T_ic^T @ WoT_ic ----
            m5_count = [0] * DC

            def emit_m5(contribs):
                for qc in range(DC):
                    ps = proj_ps.tile([P, 512], f32, name="ps_proj",
                                      tag="proj")
                    for i, (o_ap, ic) in enumerate(contribs):
                        nc.tensor.matmul(
                            ps, o_ap[:, qc * P:(qc + 1) * P],
                            wo_sb[:, ic, :],
                            start=(i == 0), stop=(i == len(contribs) - 1))
                    if m5_count[qc] == 0:
                        nc.vector.tensor_add(y_acc[:, qc, :], ps, bo_bc)
                    else:
                        nc.vector.tensor_add(y_acc[:, qc, :],
                                             y_acc[:, qc, :], ps)
                    m5_count[qc] += len(contribs)
                    if m5_count[qc] == DC:
                        eng = nc.sync if qc % 2 == 0 else nc.gpsimd
                        eng.dma_start(y[qc * P:(qc + 1) * P, :],
                                      y_acc[:, qc, :])

            # ---- partner-slot select, deferred past CC latency ----
            ex_outs = [None, None]

            def emit_select(p):
                exrd = work.tile([P, 2, NQH], bf, name="exrd", tag="exrd")
                nc.sync.dma_start(
                    exrd[:], ex_outs[p][:].rearrange("s p n -> p s n"))
                tsel = work.tile([P, NQH], bf, name="tsel", tag="tsel")
                nc.vector.tensor_scalar_mul(
                    tsel[:], exrd[:, 0, :], msel_sb[:, 0, None])
                nc.vector.tensor_scalar_mul(
                    po_sb[:, p, :], exrd[:, 1, :], msel_sb[:, 1, None])
                nc.vector.tensor_add(po_sb[:, p, :], po_sb[:, p, :],
                                     tsel[:])

            # ---- normalize split: 1/denom on ACT as exp(-ln d), read
            # straight from av psum (keeps the slow DVE reciprocal out of
            # every engine's watermark chain); PE ones-broadcast + mul +
            # DMAs deferred into the next vi once rec is ready ----
            Ln = mybir.ActivationFunctionType.Ln

            def emit_recip(av_ps_tile, rec_bf, parity):
                ln_t = work.tile([P, 2, 512], f32, name="ln_t", tag="lnt",
                                 bufs=2)
                nc.scalar.activation(
                    ln_t[DIM_HEAD:DIM_HEAD + 1, parity, :],
                    av_ps_tile[DIM_HEAD:DIM_HEAD + 1, :], Ln)
                nc.scalar.activation(
                    rec_bf[DIM_HEAD:DIM_HEAD + 1, parity, :],
                    ln_t[DIM_HEAD:DIM_HEAD + 1, parity, :], Exp, scale=-1.0)

            def emit_finish(st):
                av_sb, rec_bf, p, sl = st
                for parity in (0, 1):
                    bc = proj_ps.tile([P, 512], f32, name="ps_proj",
                                      tag="proj")
                    nc.tensor.matmul(
                        bc[0:DIM_HEAD, :],
                        ones_sb[DIM_HEAD:DIM_HEAD + 1, :],
                        rec_bf[DIM_HEAD:DIM_HEAD + 1, parity, :],
                        start=True, stop=True)
                    o_tmp = work.tile([P, 512], bf, name="o_tmp", tag="otmp")
                    nc.vector.tensor_mul(o_tmp[0:DIM_HEAD, :],
                                         av_sb[0:DIM_HEAD, parity, :],
                                         bc[0:DIM_HEAD, :])
                    nc.sync.dma_start(
                        o_sb[parity * DIM_HEAD:(parity + 1) * DIM_HEAD, p,
                             sl * NQH:(sl + 1) * NQH],
                        o_tmp[0:DIM_HEAD, :])
                if sl == 0:
                    # exchange this pair's slot-A O with the partner core
                    ex_in = dram.tile([P, NQH], bf, name="ex_in", tag="exin")
                    nc.sync.dma_start(ex_in[:], o_sb[:, p, 0:NQH])
                    ex_out = dram.tile([2, P, NQH], bf, name="ex_out",
                                       tag="exout")
                    ex_outs[p] = ex_out
                    nc.gpsimd.collective_compute(
                        "AllGather", mybir.AluOpType.bypass,
                        replica_groups=CC_GROUPS,
                        ins=[ex_in.opt()], outs=[ex_out.opt()])

            # ---- virtual head-pair passes ----
            SKEW = 3
            norm_pend = []
            for vi, (p, sl) in enumerate([(0, 0), (1, 0), (0, 1), (1, 1)]):
                av0 = av_ps.tile([DIM_HEAD + 1, 512], f32, name="av0",
                                 tag="av0")
                av1 = av_ps.tile([DIM_HEAD + 1, 512], f32, name="av1",
                                 tag="av1")
                pend = []

                def emit_av(j, pT):
                    nc.tensor.matmul(
                        av0[:, :], v_sb[:, j, 2 * p, :], pT[:, 0, :],
                        start=(j == 0), stop=(j == NCHUNK - 1))
                    nc.tensor.matmul(
                        av1[:, :], v_sb[:, j, 2 * p + 1, :], pT[:, 1, :],
                        start=(j == 0), stop=(j == NCHUNK - 1))

                for j in range(NCHUNK):
                    # previous vi's normalize finish, past the boundary
                    if j == 5 and norm_pend:
                        emit_finish(norm_pend.pop(0))
                    # projection interleave (front-loaded into vi 0/1)
                    if vi == 0:
                        if j == 0:
                            emit_m1(0, 0)
                            emit_m0(0, 0)
                        elif j == 1:
                            emit_m1(0, 1)
                        elif j % 4 == 0 and 4 <= j <= 24:
                            emit_m1(0, j // 4 + 1)
                        elif j % 4 == 2:
                            emit_m1(1, j // 4)
                        if j == 9:
                            emit_m0(1, 0)
                        emit_v(j)
                    elif vi == 1:
                        if j == 9:
                            emit_m0(0, 1)
                        elif j == 19:
                            emit_m0(1, 1)
                    elif vi == 3:
                        # selects sit past the ~60us CC horizon so the
                        # in-order DVE never blocks on the collective
                        if j == 0:
                            emit_select(0)
                        elif j == 4:
                            emit_m5([(po_sb[:, 0, :], 2)])   # partner p0
                        elif j == 12:
                            emit_m5([(o_sb[:, 0, NQH:NQ], 0)])  # own p0

                    sc = sc_ps.tile([P, 2, 512], f32, name="sc", tag="sc")
                    nc.tensor.matmul(
                        sc[:, 0, :],
                        k_sb[0:DIM_HEAD, p, j * P:(j + 1) * P],
                        q_sb[0:DIM_HEAD, p, sl * NQH:(sl + 1) * NQH],
                        start=True, stop=True)
                    nc.tensor.matmul(
                        sc[:, 1, :],
                        k_sb[DIM_HEAD:P, p, j * P:(j + 1) * P],
                        q_sb[DIM_HEAD:P, p, sl * NQH:(sl + 1) * NQH],
                        start=True, stop=True)
                    pT = pTp.tile([P, 2, 512], bf, name="pT", tag="pT")
                    if need_mask:
                        nc.scalar.activation(
                            pT[:], sc[:], Exp,
                            bias=mb_sb[:, j, None], scale=0.125)
                    else:
                        nc.scalar.activation(pT[:], sc[:], Exp, scale=0.125)
                    pend.append((j, pT))
                    if len(pend) > SKEW:
                        emit_av(*pend.pop(0))
                for item in pend:
                    emit_av(*item)

                # ---- vi tail: 1/denom on ACT from psum, evict av psum ----
                rec_bf = work.tile([P, 2, 512], bf, name="rec_bf",
                                   tag="recbf", bufs=2)
                emit_recip(av0, rec_bf, 0)
                emit_recip(av1, rec_bf, 1)
                av_sb = work.tile([P, 2, 512], f32, name="av_sb", tag="avsb")
                nc.vector.tensor_copy(out=av_sb[0:DIM_HEAD, 0, :],
                                      in_=av0[0:DIM_HEAD, :])
                nc.vector.tensor_copy(out=av_sb[0:DIM_HEAD, 1, :],
                                      in_=av1[0:DIM_HEAD, :])
                norm_pend.append((av_sb, rec_bf, p, sl))

            # tail: last vi's normalize, partner-p1 select, fused final M5
            emit_finish(norm_pend.pop(0))
            emit_select(1)
            emit_m5([(o_sb[:, 1, NQH:NQ], 1), (po_sb[:, 1, :], 3)])

    nc.compile()
    return nc


def _get_program(need_mask: bool):
    if need_mask not in _PROGRAMS:
        _PROGRAMS[need_mask] = _build(need_mask)
    return _PROGRAMS[need_mask]


def _prep_inputs(x, context, mask, Wq, Wkv, Wo, bo):
    """Host-side shard + transpose + cast. Returns list of per-core in_maps."""
    x = np.asarray(x, dtype=np.float32)
    context = np.asarray(context, dtype=np.float32)
    mask = np.asarray(mask)
    Wq = np.asarray(Wq, dtype=np.float32)
    Wkv = np.asarray(Wkv, dtype=np.float32)
    Wo = np.asarray(Wo, dtype=np.float32)
    bo = np.asarray(bo, dtype=np.float32)

    need_mask = not bool(mask.all())
    WqT = np.ascontiguousarray(Wq.T).astype(BF16)      # [D, inner]
    WkT = np.ascontiguousarray(Wkv[:D].T).astype(BF16)
    WvT = np.ascontiguousarray(Wkv[D:].T).astype(BF16)
    WoT = np.ascontiguousarray(Wo.T).astype(np.float32)  # [inner, out]
    bo2 = bo.reshape(1, D)

    ctxTs = [np.ascontiguousarray(context[b].T).astype(BF16)
             for b in range(B)]
    if need_mask:
        mb = [np.where(mask[b], 0.0, -1e30).astype(np.float32)
              .reshape(NCHUNK, P).T.copy() for b in range(B)]

    in_maps = []
    for c in range(N_CORES):
        b, hh = divmod(c, 2)
        other, own = (1 - hh) * NQH, hh * NQH
        xa = np.concatenate(
            [x[b, other:other + NQH, :], x[b, own:own + NQH, :]], axis=0)
        # Wo inner chunks permuted: [my 256 rows, partner 256 rows]
        woTa = np.concatenate(
            [WoT[hh * 256:(hh + 1) * 256, :],
             WoT[(1 - hh) * 256:(2 - hh) * 256, :]], axis=0).astype(BF16)
        msel = np.zeros((P, 2), np.float32)
        msel[:, 1 - hh] = 1.0   # pick partner slot from AllGather
        m = {
            "xT": np.ascontiguousarray(xa.T).astype(BF16),
            "ctxT": ctxTs[b],
            "wqT": np.ascontiguousarray(WqT[:, hh * 256:(hh + 1) * 256]),
            "wkT": np.ascontiguousarray(WkT[:, hh * 256:(hh + 1) * 256]),
            "wvT": np.ascontiguousarray(WvT[:, hh * 256:(hh + 1) * 256]),
            "woT": woTa,
            "bo": bo2,
            "msel": msel,
        }
        if need_mask:
            m["maskb"] = mb[b]
        in_maps.append(m)
    return in_maps, need_mask


def run_sharded(inputs, trace=False):
    """Run on 8 cores; returns (full_output, BassKernelResults)."""
    from concourse import bass_utils
    in_maps, need_mask = _prep_inputs(**inputs)
    nc = _get_program(need_mask)
    res = bass_utils.run_bass_kernel_spmd(
        nc, in_maps, core_ids=list(range(N_CORES)), trace=trace)
    out = np.empty((B, NQ, D), dtype=np.float32)
    for c in range(N_CORES):
        b, hh = divmod(c, 2)
        out[b, hh * NQH:(hh + 1) * NQH, :] = res.results[c]["y"]
    return out, res


def kernel(**inputs) -> np.ndarray:
    out, _ = run_sharded(inputs, trace=False)
    return out
